# revision 1
# baseline (speedup 1.0000x reference)
"""LinkWeightDecoder Trainium2 kernel.

out[e] = MLP(concat(emb[src[e]], emb[dst[e]])) for 1M edges, sharded
data-parallel over 8 NeuronCores.

Layer 1 is linear in each endpoint, so per-node projections
  A1[u] = emb[u] @ W1[:D] + b1,   A2[u] = emb[u] @ W1[D:]
are precomputed once per node (host, 3.3 GFLOP; the standard GNN
strength reduction) and stored f32 (512B gather descriptors run at a
better per-descriptor rate than 256B ones). The device computes
  out[e] = relu(relu(A1[src] + A2[dst]) @ W2 + b2) @ W3 + b3.
This halves gather bytes vs f32 embeddings, removes the per-edge
first-layer matmuls, and the remaining MLP runs fp16 at 1 cyc/row.

Device pipeline per core, per 1024-edge batch:
  - dma_gather (SWDGE, plain mode, 4 queues round-robin: HW gathers are
    descriptor-latency-bound, ~8.6 ns/desc on one queue vs ~3.5 on four;
    transpose-mode gathers corrupt data across queues so stay plain)
    pulls A1[src] + A2[dst] 512B rows into edge-major SBUF f32
  - per 512-edge group: PE transposes 128x128 blocks to feature-major
    PSUM; DVE adds the two PSUM tiles + relu -> h1 fp16
  - PE h1@W2 -> ACT relu+b2 -> PE @W3 -> ACT copy+b3
  - outputs accumulate [1, 16*1024] f32 in SBUF, flushed as 64KB DMAs

Edges are bucketed host-side by (src>>15, dst>>15) so int16 gather
indices stay in range with per-bucket table bases; per-bucket per-core
capacity is 128-aligned (<=1.3% padding) and batches spanning bucket
boundaries issue one gather call per run. All 8 cores share one
program; padded slots gather row 0 and are dropped host-side.
"""
import math
import numpy as np

import concourse.bass as bass
import concourse.bacc as bacc
import concourse.mybir as mybir
import concourse.tile as tile
from concourse.bass_utils import run_bass_kernel_spmd

N = 100000
D = 128
E = 1000000
H1, H2 = 128, 64
NCORES = 8
RS = 32768            # node range per int16-indexed table slice
NRANGES = (N + RS - 1) // RS
BATCH = 1024          # edges per batch (SWDGE per-call descriptor limit)
GROUP = 512           # edges per matmul chunk (PSUM bank free limit)
ALIGN = 128           # per-bucket per-core capacity alignment
OUTFLUSH = 16         # batches accumulated in SBUF before output flush

f32 = mybir.dt.float32
f16 = mybir.dt.float16
i16 = mybir.dt.int16

_AF = mybir.ActivationFunctionType


def _wrap(vals):
    """[L] int16 -> [128, L//16]: pos i -> [i%16, i//16], replicated 8x
    down the partitions for the 8 Q7 cores."""
    w = vals.reshape(-1, 16).T
    return np.tile(w, (8, 1))


def _prepare(inputs):
    """Host: per-node projections + bucket/shard edges + gather-call plan."""
    emb = np.asarray(inputs["node_embeddings"], np.float32)
    W1 = np.asarray(inputs["W1"], np.float32)
    b1 = np.asarray(inputs["b1"], np.float32).reshape(-1)
    a1t = np.ascontiguousarray((emb @ W1[:D] + b1).astype(np.float32))
    a2t = np.ascontiguousarray((emb @ W1[D:]).astype(np.float32))

    ei = np.asarray(inputs["edge_index"]).astype(np.int64)
    src, dst = ei[0], ei[1]
    bucket = (src >> 15) * NRANGES + (dst >> 15)
    order = np.argsort(bucket, kind="stable")
    counts = np.bincount(bucket, minlength=NRANGES * NRANGES)

    bucket_ids, caps = [], []
    for b in range(NRANGES * NRANGES):
        if counts[b] == 0:
            continue
        per_core = math.ceil(counts[b] / NCORES)
        caps.append(math.ceil(per_core / ALIGN) * ALIGN)
        bucket_ids.append(b)
    ncap = sum(caps)

    sloc = np.zeros((NCORES, ncap), np.int16)
    dloc = np.zeros((NCORES, ncap), np.int16)
    pos2edge = np.full((NCORES, ncap), -1, np.int64)

    boundaries = np.cumsum(counts)
    base = 0
    bucket_spans = []  # (col_base, cap, src_base, src_len, dst_base, dst_len)
    for k, b in enumerate(bucket_ids):
        lo = boundaries[b] - counts[b]
        ids_all = order[lo:boundaries[b]]
        splits = np.array_split(ids_all, NCORES)
        bs, bd = b // NRANGES, b % NRANGES
        sb, db = bs << 15, bd << 15
        for c in range(NCORES):
            ids = splits[c]
            sloc[c, base: base + len(ids)] = (src[ids] - sb).astype(np.int16)
            dloc[c, base: base + len(ids)] = (dst[ids] - db).astype(np.int16)
            pos2edge[c, base: base + len(ids)] = ids
        bucket_spans.append((base, caps[k], sb, min(RS, N - sb), db,
                             min(RS, N - db)))
        base += caps[k]

    # Batches + per-batch gather calls (one per bucket-run within batch).
    nb = math.ceil(ncap / BATCH)
    tiles = []
    g16 = 0
    for t in range(nb):
        t0, t1 = t * BATCH, min(ncap, (t + 1) * BATCH)
        tcols = t1 - t0
        calls = []
        for (cb, cap, sb, sl, db, dl) in bucket_spans:
            lo, hi = max(cb, t0), min(cb + cap, t1)
            if lo < hi:
                calls.append((lo - t0, hi - lo, sb, sl, db, dl))
        tiles.append({"t0": t0, "tcols": tcols, "goff": g16, "calls": calls})
        g16 += 2 * tcols // 16

    # Per-core index image: per batch, [src wrap block | dst wrap block].
    gidx = np.zeros((NCORES, 128, g16), np.int16)
    for c in range(NCORES):
        for tl in tiles:
            t0, tcols, goff = tl["t0"], tl["tcols"], tl["goff"]
            c16 = tcols // 16
            gidx[c, :, goff: goff + c16] = _wrap(sloc[c, t0: t0 + tcols])
            gidx[c, :, goff + c16: goff + 2 * c16] = _wrap(
                dloc[c, t0: t0 + tcols])

    plan = {"ncap": ncap, "g16": g16, "tiles": tiles}
    return {"plan": plan, "gidx": gidx, "pos2edge": pos2edge,
            "a1t": a1t, "a2t": a2t}


def _build_program(plan, b3f, reps=1):
    nc = bacc.Bacc(num_swdge_queues=4)
    ncap, g16 = plan["ncap"], plan["g16"]
    a1t = nc.dram_tensor("a1t", [N, D], f32, kind="ExternalInput")
    a2t = nc.dram_tensor("a2t", [N, D], f32, kind="ExternalInput")
    gidx = nc.dram_tensor("gidx", [128, g16], i16, kind="ExternalInput")
    w2 = nc.dram_tensor("w2", [H1, H2], f16, kind="ExternalInput")
    w3 = nc.dram_tensor("w3", [H2, 1], f16, kind="ExternalInput")
    b2 = nc.dram_tensor("b2", [H2, 1], f32, kind="ExternalInput")
    ident = nc.dram_tensor("ident", [128, 128], f32, kind="ExternalInput")
    out_d = nc.dram_tensor("out", [1, ncap], f32, kind="ExternalOutput")

    with tile.TileContext(nc) as tc:
        with (
            tc.tile_pool(name="const", bufs=1) as cpool,
            tc.tile_pool(name="idx", bufs=3) as ipool,
            tc.tile_pool(name="g1", bufs=3) as g1pool,
            tc.tile_pool(name="g2", bufs=3) as g2pool,
            tc.tile_pool(name="h1", bufs=3) as hpool,
            tc.tile_pool(name="h2", bufs=3) as h2pool,
            tc.tile_pool(name="osb", bufs=2) as opool,
            tc.tile_pool(name="pT", bufs=3, space="PSUM") as pTp,
            tc.tile_pool(name="p2p", bufs=2, space="PSUM") as p2p,
            tc.tile_pool(name="p3p", bufs=2, space="PSUM") as p3p,
        ):
            w2_t = cpool.tile([H1, H2], f16)
            w3_t = cpool.tile([H2, 1], f16)
            b2_t = cpool.tile([H2, 1], f32)
            id_t = cpool.tile([128, 128], f32)
            nc.sync.dma_start(out=w2_t[:], in_=w2[:, :])
            nc.sync.dma_start(out=w3_t[:], in_=w3[:, :])
            nc.sync.dma_start(out=b2_t[:], in_=b2[:, :])
            nc.sync.dma_start(out=id_t[:], in_=ident[:, :])

            qctr = 0
            for _ in range(reps):
                outsb = None
                flush_lo = 0
                for bi, tl in enumerate(plan["tiles"]):
                    t0, tcols, goff = tl["t0"], tl["tcols"], tl["goff"]
                    c16 = tcols // 16
                    if outsb is None:
                        outsb = opool.tile([1, OUTFLUSH * BATCH], f32,
                                           tag="osb")
                        flush_lo = bi
                        row0 = t0
                    row = t0 - row0

                    it = ipool.tile([128, 2 * c16], i16, tag="it")
                    nc.sync.dma_start(out=it[:],
                                      in_=gidx[:, goff: goff + 2 * c16])

                    g_s = g1pool.tile([128, tcols], f32, tag="g1")
                    g_d = g2pool.tile([128, tcols], f32, tag="g2")
                    g_s3 = g_s[:].rearrange("p (j f) -> p j f", f=D)
                    g_d3 = g_d[:].rearrange("p (j f) -> p j f", f=D)
                    for (o, L, sb, sl, db, dl) in tl["calls"]:
                        nc.gpsimd.dma_gather(
                            out_ap=g_s3[:, o // 128: (o + L) // 128, :],
                            in_ap=a1t[sb: sb + sl, :],
                            idxs_ap=it[:, o // 16: (o + L) // 16],
                            num_idxs=L, num_idxs_reg=L, elem_size=D,
                            queue_num=qctr % 4,
                        )
                        qctr += 1
                    for (o, L, sb, sl, db, dl) in tl["calls"]:
                        nc.gpsimd.dma_gather(
                            out_ap=g_d3[:, o // 128: (o + L) // 128, :],
                            in_ap=a2t[db: db + dl, :],
                            idxs_ap=it[:, c16 + o // 16: c16 + (o + L) // 16],
                            num_idxs=L, num_idxs_reg=L, elem_size=D,
                            queue_num=qctr % 4,
                        )
                        qctr += 1

                    for g in range(math.ceil(tcols / GROUP)):
                        lo = g * GROUP
                        hi = min(tcols, lo + GROUP)
                        gcols = hi - lo
                        nblk = gcols // 128
                        pT = pTp.tile([128, gcols], f32, space="PSUM",
                                      tag="pT")
                        for jj in range(nblk):
                            blk = lo // 128 + jj
                            # transpose-accumulate: pT = g_s^T + g_d^T
                            nc.tensor.matmul(
                                out=pT[:, jj * 128:(jj + 1) * 128],
                                lhsT=g_s[:, blk * 128:(blk + 1) * 128],
                                rhs=id_t[:], is_transpose=True,
                                start=True, stop=False,
                            )
                            nc.tensor.matmul(
                                out=pT[:, jj * 128:(jj + 1) * 128],
                                lhsT=g_d[:, blk * 128:(blk + 1) * 128],
                                rhs=id_t[:], is_transpose=True,
                                start=False, stop=True,
                            )
                        h1 = hpool.tile([128, gcols], f16, tag="h1")
                        nc.scalar.activation(h1[:], pT[:], _AF.Relu)

                        p2 = p2p.tile([H2, gcols], f32, space="PSUM",
                                      tag="p2")
                        nc.tensor.matmul(out=p2[:], lhsT=w2_t[:], rhs=h1[:],
                                         start=True, stop=True)
                        h2s = h2pool.tile([H2, gcols], f16, tag="h2")
                        nc.scalar.activation(h2s[:], p2[:], _AF.Relu,
                                             bias=b2_t[:])
                        p3 = p3p.tile([1, gcols], f32, space="PSUM",
                                      tag="p3")
                        nc.tensor.matmul(out=p3[:], lhsT=w3_t[:], rhs=h2s[:],
                                         start=True, stop=True)
                        nc.scalar.activation(
                            outsb[0:1, row + lo: row + hi], p3[:],
                            _AF.Copy, bias=b3f)

                    if (bi - flush_lo == OUTFLUSH - 1
                            or bi == len(plan["tiles"]) - 1):
                        nc.sync.dma_start(
                            out=out_d[0:1, row0: t0 + tcols],
                            in_=outsb[0:1, : row + tcols],
                        )
                        outsb = None

    nc.compile()
    return nc


def _in_maps(prep):
    base = {
        "a1t": np.ascontiguousarray(prep["a1t"]),
        "a2t": np.ascontiguousarray(prep["a2t"]),
        "w2": np.ascontiguousarray(prep["w2"]),
        "w3": np.ascontiguousarray(prep["w3"]),
        "b2": np.ascontiguousarray(prep["b2"]),
        "ident": np.eye(128, dtype=np.float32),
    }
    return [dict(base, gidx=prep["gidx"][c]) for c in range(NCORES)]


def _build(inputs, prep=None, reps=1):
    """Compile the bass program + per-core input maps (shared with test.py)."""
    if prep is None:
        prep = _prepare(inputs)
    prep["w2"] = np.asarray(inputs["W2"], np.float32).astype(np.float16)
    prep["w3"] = np.asarray(inputs["W3"], np.float32).astype(np.float16)
    prep["b2"] = np.asarray(inputs["b2"], np.float32).reshape(H2, 1)
    b3f = float(np.asarray(inputs["b3"], np.float32).reshape(-1)[0])
    nc = _build_program(prep["plan"], b3f, reps=reps)
    maps = _in_maps(prep)
    return {"nc": nc, "maps": maps, "prep": prep}


def kernel(**inputs):
    prep = _prepare(inputs)
    built = _build(inputs, prep)
    res = run_bass_kernel_spmd(built["nc"], built["maps"],
                               list(range(NCORES)))

    pos2edge = prep["pos2edge"]
    out = np.zeros(E, np.float32)
    for c in range(NCORES):
        dev = res.results[c]["out"].reshape(-1)
        m = pos2edge[c] >= 0
        out[pos2edge[c][m]] = dev[m]
    return out.reshape(E, 1)



# revision 14
# speedup vs baseline: 1.3752x; 1.3752x over previous
"""LinkWeightDecoder Trainium2 kernel (v2).

out[e] = MLP(concat(emb[src[e]], emb[dst[e]])) for 1M edges over 8 cores.

Layer 1 is linear per endpoint, so per-node projections
  A1[u] = emb[u] @ W1[:D] + b1,   A2[u] = emb[u] @ W1[D:]
are precomputed per node (host, O(N*D*H1)) and stored f16. The device
computes out[e] = relu(relu(A1[src]+A2[dst]) @ W2 + b2) @ W3 + b3.

v2 design (vs v1's two per-edge gather sides): the HW floor is the SDMA
descriptor cost (~2.9 ns/desc measured, byte-count nearly irrelevant at
256B rows), so only the dst side pays per-edge descriptors:

- Edges shard by src block (12544 nodes/core), sort by (dst_bucket, src).
- src side has ZERO descriptors: for each 256-slot halfgroup the host
  streams a [64,128] stationary (the halfgroup's <=64 distinct nodes'
  A1 rows) plus a [64,256] one-hot routing matrix; one f16 matmul
  produces A1[src] feature-major in PSUM. 192B/slot of contiguous
  stream replaces a 2.9ns descriptor per slot.
- dst side: per-edge dma_gather of A2 rows (256B f16), 4 queues. int16
  indices are kept in range by 4 dst-bucket mega-runs per core (25000
  nodes/bucket), run capacities equalized across cores so all 8 cores
  share one program. Gathered edge-major tiles are transposed into the
  same PSUM group by f16 identity matmuls (regular matmul lhsT^T @ I,
  accumulating in f32).
- MLP: ACT relu -> h1 f16; W2 matmul; DVE fused (+b2, relu) -> h2 f16;
  W3 matmuls write [1,512] rows at PSUM partitions {0,32,64,96} via
  tile_position so output copies run 4 groups at a time.

Pad slots (run alignment + capacity equalization) gather row 0 / zero
one-hot columns and are dropped host-side via pos2edge.
"""
import math
import numpy as np

import concourse.bass as bass
import concourse.bacc as bacc
import concourse.mybir as mybir
import concourse.tile as tile
from concourse.bass_utils import run_bass_kernel_spmd

N = 100000
D = 128
E = 1000000
H1, H2 = 128, 64
NCORES = 8

NPC = 12544           # nodes per core (64-aligned, 8*NPC >= N)
DB = 25000            # dst bucket width (int16-safe indices)
NBUCK = 4
GROUP = 512           # slots per PSUM group
HG = 128              # slots per halfgroup (stationary unit)
BATCH = 2048          # slots per gather batch / gd tile
CALLMAX = 1024        # max idxs per dma_gather call
RUNALIGN = 128
OUTTILE = 8           # groups per output flush tile (4096 slots)
STREAMB = 2           # batches per stream DMA (4096 slots)
IDXB = 8              # batches per idx DMA

STB = 256             # stream cols per halfgroup: 128 stationary + 128 onehot

f32 = mybir.dt.float32
f16 = mybir.dt.float16
i16 = mybir.dt.int16

_AF = mybir.ActivationFunctionType
_ALU = mybir.AluOpType


def _wrap(vals):
    """[L] int16 -> [128, L//16]: pos i -> [i%16, i//16], replicated 8x
    down the partitions for the 8 Q7 cores."""
    w = vals.reshape(-1, 16).T
    return np.tile(w, (8, 1))


def _prepare(inputs):
    emb = np.asarray(inputs["node_embeddings"], np.float32)
    W1 = np.asarray(inputs["W1"], np.float32)
    b1 = np.asarray(inputs["b1"], np.float32).reshape(-1)
    a1 = (emb @ W1[:D] + b1).astype(np.float16)
    a2 = (emb @ W1[D:]).astype(np.float16)

    ei = np.asarray(inputs["edge_index"]).astype(np.int64)
    src, dst = ei[0], ei[1]
    core = np.minimum(src // NPC, NCORES - 1)

    # per-core edge lists sorted by (dst bucket, src)
    per_core = []
    counts = np.zeros((NCORES, NBUCK), np.int64)
    for c in range(NCORES):
        m = np.where(core == c)[0]
        es, ed = src[m], dst[m]
        bucket = ed // DB
        order = np.lexsort((es, bucket))
        m, es, ed, bucket = m[order], es[order], ed[order], bucket[order]
        for b in range(NBUCK):
            counts[c, b] = int((bucket == b).sum())
        per_core.append((m, es, ed, bucket))

    caps = [int(math.ceil(counts[:, b].max() / RUNALIGN) * RUNALIGN)
            for b in range(NBUCK)]
    C0 = sum(caps)
    C = int(math.ceil(C0 / (OUTTILE * GROUP)) * (OUTTILE * GROUP))
    tailpad = C - C0
    runs = [(sum(caps[:b]), caps[b], b) for b in range(NBUCK)]
    if tailpad:
        runs.append((C0, tailpad, 0))

    # slot arrays
    ssrc = np.full((NCORES, C), -1, np.int64)     # -1 = pad
    sdst16 = np.zeros((NCORES, C), np.int16)
    pos2edge = np.full((NCORES, C), -1, np.int64)
    for c in range(NCORES):
        m, es, ed, bucket = per_core[c]
        for b in range(NBUCK):
            lo = int(np.searchsorted(bucket, b))
            hi = int(np.searchsorted(bucket, b + 1))
            s0 = sum(caps[:b])
            n = hi - lo
            ssrc[c, s0:s0 + n] = es[lo:hi]
            sdst16[c, s0:s0 + n] = (ed[lo:hi] - b * DB).astype(np.int16)
            pos2edge[c, s0:s0 + n] = m[lo:hi]

    # gather call plan (common to all cores): (batch, off_in_batch, L, b)
    calls_by_batch = [[] for _ in range(C // BATCH)]
    for (r0, rlen, b) in runs:
        cur = r0
        end = r0 + rlen
        while cur < end:
            nb = (cur // BATCH + 1) * BATCH
            L = min(CALLMAX, end - cur, nb - cur)
            calls_by_batch[cur // BATCH].append((cur % BATCH, L, b))
            cur += L

    # idx image: global wrap of sdst16 (call slices line up since every
    # call offset is 128-aligned)
    gidx = np.zeros((NCORES, 128, C // 16), np.int16)
    for c in range(NCORES):
        gidx[c] = _wrap(sdst16[c])

    # stream image: per halfgroup [64, 128 stationary | 128 onehot].
    # Halfgroups with >64 distinct src nodes (rare) keep the 64 busiest
    # nodes; dropped slots are routed to the host fixup path.
    nhg = C // HG
    stream = np.zeros((NCORES, 64, nhg * STB), np.float16)
    fixup_edges = []
    for c in range(NCORES):
        sc = ssrc[c]
        for h in range(nhg):
            seg = sc[h * HG:(h + 1) * HG]
            valid = seg >= 0
            if not valid.any():
                continue
            nodes, inv, cnt = np.unique(seg[valid], return_inverse=True,
                                        return_counts=True)
            cols = np.nonzero(valid)[0]
            if len(nodes) > 64:
                keep = np.sort(np.argsort(-cnt, kind="stable")[:64])
                kept_mask = np.isin(inv, keep)
                drop_cols = cols[~kept_mask]
                drop_slots = h * HG + drop_cols
                fixup_edges.extend(pos2edge[c, drop_slots].tolist())
                pos2edge[c, drop_slots] = -1
                remap = -np.ones(len(nodes), np.int64)
                remap[keep] = np.arange(64)
                nodes = nodes[keep]
                inv = remap[inv]
                cols = cols[kept_mask]
                inv = inv[kept_mask]
            blk = stream[c, :, h * STB:(h + 1) * STB]
            blk[:len(nodes), :128] = a1[nodes]
            blk[inv, 128 + cols] = np.float16(1.0)

    plan = {"C": C, "calls_by_batch": calls_by_batch}
    return {"plan": plan, "gidx": gidx, "stream": stream,
            "pos2edge": pos2edge, "a2": a2,
            "fixup_edges": np.array(sorted(fixup_edges), np.int64)}


def _build_program(plan, b3f, reps=1, dbg_groups=0):
    nc = bacc.Bacc(num_swdge_queues=4)
    C = plan["C"]
    nhg = C // HG
    dbg_d = dbg2_d = None
    if dbg_groups:
        dbg_d = nc.dram_tensor("dbg", [128, dbg_groups * GROUP], f16,
                               kind="ExternalOutput")
        dbg2_d = nc.dram_tensor("dbg2", [H2, dbg_groups * GROUP], f16,
                                kind="ExternalOutput")
    a2t = nc.dram_tensor("a2t", [N, D], f16, kind="ExternalInput")
    gidx = nc.dram_tensor("gidx", [128, C // 16], i16, kind="ExternalInput")
    stream_d = nc.dram_tensor("stream", [64, nhg * STB], f16,
                              kind="ExternalInput")
    w2 = nc.dram_tensor("w2", [H1, H2], f16, kind="ExternalInput")
    w3 = nc.dram_tensor("w3", [H2, 1], f16, kind="ExternalInput")
    b2 = nc.dram_tensor("b2", [H2, 1], f32, kind="ExternalInput")
    ident = nc.dram_tensor("ident", [128, 128], f16, kind="ExternalInput")
    out_d = nc.dram_tensor("out", [4, C // 4], f16, kind="ExternalOutput")

    nbatch = C // BATCH
    g_per_b = BATCH // GROUP          # 4
    hg_per_g = GROUP // HG            # 2
    b_per_ot = OUTTILE * GROUP // BATCH   # 4 batches per outtile

    with tile.TileContext(nc) as tc:
        with (
            tc.tile_pool(name="const", bufs=1) as cpool,
            tc.tile_pool(name="idx", bufs=3) as ipool,
            tc.tile_pool(name="stm", bufs=3) as spool,
            tc.tile_pool(name="gd", bufs=6) as gpool,
            tc.tile_pool(name="h1", bufs=3) as h1pool,
            tc.tile_pool(name="h2", bufs=3) as h2pool,
            tc.tile_pool(name="osb", bufs=2) as opool,
            tc.tile_pool(name="pT", bufs=3, space="PSUM") as pTp,
            tc.tile_pool(name="p2p", bufs=2, space="PSUM") as p2p,
            tc.tile_pool(name="p3p", bufs=2, space="PSUM") as p3p,
        ):
            w2_t = cpool.tile([H1, H2], f16)
            w3_t = cpool.tile([H2, 1], f16)
            b2_t = cpool.tile([H2, 1], f32)
            id_t = cpool.tile([128, 128], f16)
            nc.sync.dma_start(out=w2_t[:], in_=w2[:, :])
            nc.sync.dma_start(out=w3_t[:], in_=w3[:, :])
            nc.sync.dma_start(out=b2_t[:], in_=b2[:, :])
            nc.sync.dma_start(out=id_t[:], in_=ident[:, :])

            qctr = 0
            for _ in range(reps):
                it = st = outsb = p3 = None
                for bi in range(nbatch):
                    s0 = bi * BATCH
                    if bi % IDXB == 0:
                        icols = (min(C, s0 + IDXB * BATCH) - s0) // 16
                        it = ipool.tile([128, IDXB * BATCH // 16], i16,
                                        tag="it")
                        nc.sync.dma_start(
                            out=it[:, :icols],
                            in_=gidx[:, s0 // 16:s0 // 16 + icols])
                    if bi % STREAMB == 0:
                        scols = STREAMB * (BATCH // HG) * STB
                        soff = (s0 // HG) * STB
                        st = spool.tile([64, scols], f16, tag="st")
                        nc.sync.dma_start(
                            out=st[:], in_=stream_d[:, soff:soff + scols])
                    if bi % b_per_ot == 0:
                        outsb = opool.tile([128, OUTTILE * GROUP // 4], f16,
                                           tag="osb")

                    gd = gpool.tile([128, BATCH], f16, tag="gd")
                    gd3 = gd[:].rearrange("p (j f) -> p j f", f=D)
                    for (off, L, b) in plan["calls_by_batch"][bi]:
                        blo = b * DB
                        bhi = min(N, blo + DB)
                        icol = (s0 + off - (bi // IDXB) * IDXB * BATCH) // 16
                        nc.gpsimd.dma_gather(
                            out_ap=gd3[:, off // 128:(off + L) // 128, :],
                            in_ap=a2t[blo:bhi, :],
                            idxs_ap=it[:, icol:icol + L // 16],
                            num_idxs=L, num_idxs_reg=L, elem_size=D,
                            queue_num=qctr % 4,
                        )
                        qctr += 1

                    for g in range(g_per_b):
                        gg = bi * g_per_b + g          # global group idx
                        pT = pTp.tile([128, GROUP], f32, space="PSUM",
                                      tag="pT")
                        # src: one-hot matmuls, one per halfgroup
                        for k in range(hg_per_g):
                            hg = (s0 + g * GROUP) // HG + k
                            sb = (hg - (bi - bi % STREAMB) * BATCH // HG) \
                                * STB
                            nc.tensor.matmul(
                                out=pT[:, k * HG:(k + 1) * HG],
                                lhsT=st[:, sb:sb + 128],
                                rhs=st[:, sb + 128:sb + STB],
                                start=(k == 0), stop=False)
                        # dst: identity-matmul transposes, accumulate
                        for j in range(GROUP // 128):
                            blk = g * (GROUP // 128) + j
                            nc.tensor.matmul(
                                out=pT[:, j * 128:(j + 1) * 128],
                                lhsT=gd[:, blk * 128:(blk + 1) * 128],
                                rhs=id_t[:],
                                start=False, stop=(j == GROUP // 128 - 1))

                        h1 = h1pool.tile([128, GROUP], f16, tag="h1")
                        nc.scalar.activation(h1[:], pT[:], _AF.Relu)
                        if dbg_d is not None and gg < dbg_groups:
                            nc.sync.dma_start(
                                out=dbg_d[:, gg * GROUP:(gg + 1) * GROUP],
                                in_=h1[:])
                        p2 = p2p.tile([H2, GROUP], f32, space="PSUM",
                                      tag="p2")
                        nc.tensor.matmul(out=p2[:], lhsT=w2_t[:], rhs=h1[:],
                                         start=True, stop=True)
                        h2s = h2pool.tile([H2, GROUP], f16, tag="h2")
                        nc.vector.tensor_scalar(
                            out=h2s[:], in0=p2[:], scalar1=b2_t[:],
                            scalar2=0.0, op0=_ALU.add, op1=_ALU.max)
                        if dbg2_d is not None and gg < dbg_groups:
                            nc.sync.dma_start(
                                out=dbg2_d[:, gg * GROUP:(gg + 1) * GROUP],
                                in_=h2s[:])
                        q = gg % 4
                        if q == 0:
                            p3 = p3p.tile([128, GROUP], f32, space="PSUM",
                                          tag="p3")
                        nc.tensor.matmul(out=p3[32 * q:32 * q + 1, :],
                                         lhsT=w3_t[:], rhs=h2s[:],
                                         start=True, stop=True,
                                         tile_position=(0, 32 * q),
                                         skip_group_check=True)
                        if q == 3:
                            k4 = (gg // 4) % (OUTTILE // 4)
                            nc.scalar.activation(
                                outsb[:, k4 * GROUP:(k4 + 1) * GROUP],
                                p3[:], _AF.Copy, bias=b3f)

                    if (bi + 1) % b_per_ot == 0:
                        ot = bi // b_per_ot
                        ocols = OUTTILE * GROUP // 4
                        for qq in range(4):
                            nc.sync.dma_start(
                                out=out_d[qq:qq + 1,
                                          ot * ocols:(ot + 1) * ocols],
                                in_=outsb[32 * qq:32 * qq + 1, :])

    nc.compile()
    return nc


def _in_maps(prep):
    base = {
        "a2t": np.ascontiguousarray(prep["a2"]),
        "w2": prep["w2"], "w3": prep["w3"], "b2": prep["b2"],
        "ident": np.eye(128, dtype=np.float16),
    }
    return [dict(base, gidx=np.ascontiguousarray(prep["gidx"][c]),
                 stream=np.ascontiguousarray(prep["stream"][c]))
            for c in range(NCORES)]


def _build(inputs, prep=None, reps=1):
    if prep is None:
        prep = _prepare(inputs)
    prep["w2"] = np.asarray(inputs["W2"], np.float32).astype(np.float16)
    prep["w3"] = np.asarray(inputs["W3"], np.float32).astype(np.float16)
    prep["b2"] = np.asarray(inputs["b2"], np.float32).reshape(H2, 1)
    b3f = float(np.asarray(inputs["b3"], np.float32).reshape(-1)[0])
    nc = _build_program(prep["plan"], b3f, reps=reps)
    maps = _in_maps(prep)
    return {"nc": nc, "maps": maps, "prep": prep}


def _slot_of_out(C):
    """slot index for each element of the [4, C//4] device output."""
    cols = np.arange(C // 4)
    t = cols // (OUTTILE * GROUP // 4)
    rem = cols % (OUTTILE * GROUP // 4)
    k = rem // GROUP
    cc = rem % GROUP
    # group = t*OUTTILE + k*4 + q ; slot = group*GROUP + cc
    return ((t * OUTTILE + k * 4)[None, :] + np.arange(4)[:, None]) \
        * GROUP + cc[None, :]


def kernel(**inputs):
    prep = _prepare(inputs)
    built = _build(inputs, prep)
    res = run_bass_kernel_spmd(built["nc"], built["maps"],
                               list(range(NCORES)))

    C = prep["plan"]["C"]
    slot_of = _slot_of_out(C)
    pos2edge = prep["pos2edge"]
    out = np.zeros(E, np.float32)
    for c in range(NCORES):
        dev = np.asarray(res.results[c]["out"], np.float32)  # [4, C//4]
        full = np.empty(C, np.float32)
        full[slot_of.reshape(-1)] = dev.reshape(-1)
        m = pos2edge[c] >= 0
        out[pos2edge[c][m]] = full[m]

    fix = prep["fixup_edges"]
    if len(fix):
        emb = np.asarray(inputs["node_embeddings"], np.float32)
        W1 = np.asarray(inputs["W1"], np.float32)
        b1 = np.asarray(inputs["b1"], np.float32).reshape(-1)
        ei = np.asarray(inputs["edge_index"]).astype(np.int64)
        s, d = ei[0][fix], ei[1][fix]
        h = np.maximum(emb[s] @ W1[:D] + emb[d] @ W1[D:] + b1, 0.0)
        h = np.maximum(h @ np.asarray(inputs["W2"], np.float32)
                       + np.asarray(inputs["b2"], np.float32).reshape(-1),
                       0.0)
        out[fix] = (h @ np.asarray(inputs["W3"], np.float32)).reshape(-1) \
            + float(np.asarray(inputs["b3"], np.float32).reshape(-1)[0])
    return out.reshape(E, 1)


# revision 19
# speedup vs baseline: 1.3837x; 1.0062x over previous
"""LinkWeightDecoder Trainium2 kernel (v2).

out[e] = MLP(concat(emb[src[e]], emb[dst[e]])) for 1M edges over 8 cores.

Layer 1 is linear per endpoint, so per-node projections
  A1[u] = emb[u] @ W1[:D] + b1,   A2[u] = emb[u] @ W1[D:]
are precomputed per node (host, O(N*D*H1)) and stored f16. The device
computes out[e] = relu(relu(A1[src]+A2[dst]) @ W2 + b2) @ W3 + b3.

v2 design (vs v1's two per-edge gather sides): the HW floor is the SDMA
descriptor cost (~2.9 ns/desc measured, byte-count nearly irrelevant at
256B rows), so only the dst side pays per-edge descriptors:

- Edges shard by src block (12544 nodes/core), sort by (dst_bucket, src).
- src side has ZERO descriptors: for each 256-slot halfgroup the host
  streams a [64,128] stationary (the halfgroup's <=64 distinct nodes'
  A1 rows) plus a [64,256] one-hot routing matrix; one f16 matmul
  produces A1[src] feature-major in PSUM. 192B/slot of contiguous
  stream replaces a 2.9ns descriptor per slot.
- dst side: per-edge dma_gather of A2 rows (256B f16), 4 queues. int16
  indices are kept in range by 4 dst-bucket mega-runs per core (25000
  nodes/bucket), run capacities equalized across cores so all 8 cores
  share one program. Gathered edge-major tiles are transposed into the
  same PSUM group by f16 identity matmuls (regular matmul lhsT^T @ I,
  accumulating in f32).
- MLP: ACT relu -> h1 f16; W2 matmul; DVE fused (+b2, relu) -> h2 f16;
  W3 matmuls write [1,512] rows at PSUM partitions {0,32,64,96} via
  tile_position so output copies run 4 groups at a time.

Pad slots (run alignment + capacity equalization) gather row 0 / zero
one-hot columns and are dropped host-side via pos2edge.
"""
import math
import numpy as np

import concourse.bass as bass
import concourse.bacc as bacc
import concourse.mybir as mybir
import concourse.tile as tile
from concourse.bass_utils import run_bass_kernel_spmd

N = 100000
D = 128
E = 1000000
H1, H2 = 128, 64
NCORES = 8

NPC = 12544           # nodes per core (64-aligned, 8*NPC >= N)
DB = 25000            # dst bucket width (int16-safe indices)
NBUCK = 4
GROUP = 512           # slots per PSUM group
HG = 128              # slots per halfgroup (stationary unit)
BATCH = 2048          # slots per gather batch / gd tile
CALLMAX = 1024        # max idxs per dma_gather call
RUNALIGN = 128
OUTTILE = 8           # groups per output flush tile (4096 slots)
STREAMB = 2           # batches per stream DMA (4096 slots)
IDXB = 8              # batches per idx DMA

STB = 256             # stream cols per halfgroup: 128 stationary + 128 onehot

f32 = mybir.dt.float32
f16 = mybir.dt.float16
i16 = mybir.dt.int16

_AF = mybir.ActivationFunctionType
_ALU = mybir.AluOpType


def _wrap(vals):
    """[L] int16 -> [128, L//16]: pos i -> [i%16, i//16], replicated 8x
    down the partitions for the 8 Q7 cores."""
    w = vals.reshape(-1, 16).T
    return np.tile(w, (8, 1))


def _prepare(inputs):
    emb = np.asarray(inputs["node_embeddings"], np.float32)
    W1 = np.asarray(inputs["W1"], np.float32)
    b1 = np.asarray(inputs["b1"], np.float32).reshape(-1)
    a1 = (emb @ W1[:D] + b1).astype(np.float16)
    a2 = (emb @ W1[D:]).astype(np.float16)

    ei = np.asarray(inputs["edge_index"]).astype(np.int64)
    src, dst = ei[0], ei[1]
    core = np.minimum(src // NPC, NCORES - 1)

    # per-core edge lists sorted by (dst bucket, src)
    per_core = []
    counts = np.zeros((NCORES, NBUCK), np.int64)
    for c in range(NCORES):
        m = np.where(core == c)[0]
        es, ed = src[m], dst[m]
        bucket = ed // DB
        order = np.lexsort((es, bucket))
        m, es, ed, bucket = m[order], es[order], ed[order], bucket[order]
        for b in range(NBUCK):
            counts[c, b] = int((bucket == b).sum())
        per_core.append((m, es, ed, bucket))

    caps = [int(math.ceil(counts[:, b].max() / RUNALIGN) * RUNALIGN)
            for b in range(NBUCK)]
    C0 = sum(caps)
    C = int(math.ceil(C0 / (OUTTILE * GROUP)) * (OUTTILE * GROUP))
    tailpad = C - C0
    runs = [(sum(caps[:b]), caps[b], b) for b in range(NBUCK)]
    if tailpad:
        runs.append((C0, tailpad, 0))

    # slot arrays
    ssrc = np.full((NCORES, C), -1, np.int64)     # -1 = pad
    sdst16 = np.zeros((NCORES, C), np.int16)
    pos2edge = np.full((NCORES, C), -1, np.int64)
    for c in range(NCORES):
        m, es, ed, bucket = per_core[c]
        for b in range(NBUCK):
            lo = int(np.searchsorted(bucket, b))
            hi = int(np.searchsorted(bucket, b + 1))
            s0 = sum(caps[:b])
            n = hi - lo
            ssrc[c, s0:s0 + n] = es[lo:hi]
            sdst16[c, s0:s0 + n] = (ed[lo:hi] - b * DB).astype(np.int16)
            pos2edge[c, s0:s0 + n] = m[lo:hi]

    # gather call plan (common to all cores): (batch, off_in_batch, L, b)
    calls_by_batch = [[] for _ in range(C // BATCH)]
    for (r0, rlen, b) in runs:
        cur = r0
        end = r0 + rlen
        while cur < end:
            nb = (cur // BATCH + 1) * BATCH
            L = min(CALLMAX, end - cur, nb - cur)
            calls_by_batch[cur // BATCH].append((cur % BATCH, L, b))
            cur += L

    # idx image: global wrap of sdst16 (call slices line up since every
    # call offset is 128-aligned)
    gidx = np.zeros((NCORES, 128, C // 16), np.int16)
    for c in range(NCORES):
        gidx[c] = _wrap(sdst16[c])

    # stream image: per halfgroup [64, 128 stationary | 128 onehot].
    # Halfgroups with >64 distinct src nodes (rare) keep the 64 busiest
    # nodes; dropped slots are routed to the host fixup path.
    nhg = C // HG
    stream = np.zeros((NCORES, 64, nhg * STB), np.float16)
    fixup_edges = []
    for c in range(NCORES):
        sc = ssrc[c]
        for h in range(nhg):
            seg = sc[h * HG:(h + 1) * HG]
            valid = seg >= 0
            if not valid.any():
                continue
            nodes, inv, cnt = np.unique(seg[valid], return_inverse=True,
                                        return_counts=True)
            cols = np.nonzero(valid)[0]
            if len(nodes) > 64:
                keep = np.sort(np.argsort(-cnt, kind="stable")[:64])
                kept_mask = np.isin(inv, keep)
                drop_cols = cols[~kept_mask]
                drop_slots = h * HG + drop_cols
                fixup_edges.extend(pos2edge[c, drop_slots].tolist())
                pos2edge[c, drop_slots] = -1
                remap = -np.ones(len(nodes), np.int64)
                remap[keep] = np.arange(64)
                nodes = nodes[keep]
                inv = remap[inv]
                cols = cols[kept_mask]
                inv = inv[kept_mask]
            blk = stream[c, :, h * STB:(h + 1) * STB]
            blk[:len(nodes), :128] = a1[nodes]
            blk[inv, 128 + cols] = np.float16(1.0)

    plan = {"C": C, "calls_by_batch": calls_by_batch}
    return {"plan": plan, "gidx": gidx, "stream": stream,
            "pos2edge": pos2edge, "a2": a2,
            "fixup_edges": np.array(sorted(fixup_edges), np.int64)}


def _build_program(plan, b3f, reps=1, dbg_groups=0):
    nc = bacc.Bacc(num_swdge_queues=4)
    C = plan["C"]
    nhg = C // HG
    dbg_d = dbg2_d = None
    if dbg_groups:
        dbg_d = nc.dram_tensor("dbg", [128, dbg_groups * GROUP], f16,
                               kind="ExternalOutput")
        dbg2_d = nc.dram_tensor("dbg2", [H2, dbg_groups * GROUP], f16,
                                kind="ExternalOutput")
    a2t = nc.dram_tensor("a2t", [N, D], f16, kind="ExternalInput")
    gidx = nc.dram_tensor("gidx", [128, C // 16], i16, kind="ExternalInput")
    stream_d = nc.dram_tensor("stream", [64, nhg * STB], f16,
                              kind="ExternalInput")
    w2 = nc.dram_tensor("w2", [H1, H2], f16, kind="ExternalInput")
    w3 = nc.dram_tensor("w3", [H2, 1], f16, kind="ExternalInput")
    b2 = nc.dram_tensor("b2", [H2, 1], f32, kind="ExternalInput")
    ident = nc.dram_tensor("ident", [128, 128], f16, kind="ExternalInput")
    out_d = nc.dram_tensor("out", [4, C // 4], f16, kind="ExternalOutput")

    nbatch = C // BATCH
    g_per_b = BATCH // GROUP          # 4
    hg_per_g = GROUP // HG            # 2
    b_per_ot = OUTTILE * GROUP // BATCH   # 4 batches per outtile

    with tile.TileContext(nc) as tc:
        with (
            tc.tile_pool(name="const", bufs=1) as cpool,
            tc.tile_pool(name="idx", bufs=3) as ipool,
            tc.tile_pool(name="stm", bufs=3) as spool,
            tc.tile_pool(name="gd", bufs=6) as gpool,
            tc.tile_pool(name="h1", bufs=4) as h1pool,
            tc.tile_pool(name="h2", bufs=4) as h2pool,
            tc.tile_pool(name="osb", bufs=2) as opool,
            tc.tile_pool(name="pT", bufs=4, space="PSUM") as pTp,
            tc.tile_pool(name="p2p", bufs=2, space="PSUM") as p2p,
            tc.tile_pool(name="p3p", bufs=2, space="PSUM") as p3p,
        ):
            w2_t = cpool.tile([H1, H2], f16)
            w3_t = cpool.tile([H2, 1], f16)
            b2_t = cpool.tile([H2, 1], f32)
            id_t = cpool.tile([128, 128], f16)
            nc.sync.dma_start(out=w2_t[:], in_=w2[:, :])
            nc.sync.dma_start(out=w3_t[:], in_=w3[:, :])
            nc.sync.dma_start(out=b2_t[:], in_=b2[:, :])
            nc.sync.dma_start(out=id_t[:], in_=ident[:, :])

            qctr = 0
            for _ in range(reps):
                it = st = None
                state = {"outsb": None, "p3": None}
                b1q, b2q = [], []

                def stage_b1(gg, h1):
                    p2 = p2p.tile([H2, GROUP], f32, space="PSUM", tag="p2")
                    nc.tensor.matmul(out=p2[:], lhsT=w2_t[:], rhs=h1[:],
                                     start=True, stop=True)
                    h2s = h2pool.tile([H2, GROUP], f16, tag="h2")
                    nc.vector.tensor_scalar(
                        out=h2s[:], in0=p2[:], scalar1=b2_t[:],
                        scalar2=0.0, op0=_ALU.add, op1=_ALU.max)
                    if dbg2_d is not None and gg < dbg_groups:
                        nc.sync.dma_start(
                            out=dbg2_d[:, gg * GROUP:(gg + 1) * GROUP],
                            in_=h2s[:])
                    return h2s

                def stage_b2(gg, h2s):
                    q = gg % 4
                    if q == 0:
                        p3_t = p3p.tile([128, GROUP], f32, space="PSUM", tag="p3")
                        state["p3"] = p3_t
                    p3 = state["p3"]
                    nc.tensor.matmul(out=p3[32 * q:32 * q + 1, :],
                                     lhsT=w3_t[:], rhs=h2s[:],
                                     start=True, stop=True,
                                     tile_position=(0, 32 * q),
                                     skip_group_check=True)
                    if q == 3:
                        if state["outsb"] is None:
                            osb_t = opool.tile([128, OUTTILE * GROUP // 4], f16,
                                               tag="osb")
                            state["outsb"] = osb_t
                        k4 = (gg // 4) % (OUTTILE // 4)
                        nc.scalar.activation(
                            state["outsb"][:, k4 * GROUP:(k4 + 1) * GROUP],
                            p3[:], _AF.Copy, bias=b3f)
                        if k4 == OUTTILE // 4 - 1:
                            ot = gg // OUTTILE
                            ocols = OUTTILE * GROUP // 4
                            for qq in range(4):
                                nc.sync.dma_start(
                                    out=out_d[qq:qq + 1,
                                              ot * ocols:(ot + 1) * ocols],
                                    in_=state["outsb"][32 * qq:32 * qq + 1,
                                                       :])
                            state["outsb"] = None

                def pump(b1q, b2q):
                    if len(b2q) > 1:
                        gg2, h2s = b2q.pop(0)
                        stage_b2(gg2, h2s)
                    if len(b1q) > 1:
                        gg1, h1 = b1q.pop(0)
                        b2q.append((gg1, stage_b1(gg1, h1)))

                for bi in range(nbatch):
                    s0 = bi * BATCH
                    if bi % IDXB == 0:
                        icols = (min(C, s0 + IDXB * BATCH) - s0) // 16
                        it = ipool.tile([128, IDXB * BATCH // 16], i16,
                                        tag="it")
                        nc.sync.dma_start(
                            out=it[:, :icols],
                            in_=gidx[:, s0 // 16:s0 // 16 + icols])
                    if bi % STREAMB == 0:
                        scols = STREAMB * (BATCH // HG) * STB
                        soff = (s0 // HG) * STB
                        st = spool.tile([64, scols], f16, tag="st")
                        nc.sync.dma_start(
                            out=st[:], in_=stream_d[:, soff:soff + scols])
                    gd = gpool.tile([128, BATCH], f16, tag="gd")
                    gd3 = gd[:].rearrange("p (j f) -> p j f", f=D)
                    for (off, L, b) in plan["calls_by_batch"][bi]:
                        blo = b * DB
                        bhi = min(N, blo + DB)
                        icol = (s0 + off - (bi // IDXB) * IDXB * BATCH) // 16
                        nc.gpsimd.dma_gather(
                            out_ap=gd3[:, off // 128:(off + L) // 128, :],
                            in_ap=a2t[blo:bhi, :],
                            idxs_ap=it[:, icol:icol + L // 16],
                            num_idxs=L, num_idxs_reg=L, elem_size=D,
                            queue_num=qctr % 4,
                        )
                        qctr += 1

                    for g in range(g_per_b):
                        gg = bi * g_per_b + g          # global group idx
                        pT = pTp.tile([128, GROUP], f32, space="PSUM",
                                      tag="pT")
                        # src: one-hot matmuls, one per halfgroup
                        for k in range(hg_per_g):
                            hg = (s0 + g * GROUP) // HG + k
                            sb = (hg - (bi - bi % STREAMB) * BATCH // HG) \
                                * STB
                            nc.tensor.matmul(
                                out=pT[:, k * HG:(k + 1) * HG],
                                lhsT=st[:, sb:sb + 128],
                                rhs=st[:, sb + 128:sb + STB],
                                start=(k == 0), stop=False)
                        # dst: identity-matmul transposes, accumulate
                        for j in range(GROUP // 128):
                            blk = g * (GROUP // 128) + j
                            nc.tensor.matmul(
                                out=pT[:, j * 128:(j + 1) * 128],
                                lhsT=gd[:, blk * 128:(blk + 1) * 128],
                                rhs=id_t[:],
                                start=False, stop=(j == GROUP // 128 - 1))

                        h1 = h1pool.tile([128, GROUP], f16, tag="h1")
                        nc.scalar.activation(h1[:], pT[:], _AF.Relu)
                        if dbg_d is not None and gg < dbg_groups:
                            nc.sync.dma_start(
                                out=dbg_d[:, gg * GROUP:(gg + 1) * GROUP],
                                in_=h1[:])
                        b1q.append((gg, h1))
                        pump(b1q, b2q)

                # drain the software pipeline
                while b1q or b2q:
                    if b2q:
                        gg2, h2s = b2q.pop(0)
                        stage_b2(gg2, h2s)
                    if b1q:
                        gg1, h1 = b1q.pop(0)
                        b2q.append((gg1, stage_b1(gg1, h1)))

    nc.compile()
    return nc


def _in_maps(prep):
    base = {
        "a2t": np.ascontiguousarray(prep["a2"]),
        "w2": prep["w2"], "w3": prep["w3"], "b2": prep["b2"],
        "ident": np.eye(128, dtype=np.float16),
    }
    return [dict(base, gidx=np.ascontiguousarray(prep["gidx"][c]),
                 stream=np.ascontiguousarray(prep["stream"][c]))
            for c in range(NCORES)]


def _build(inputs, prep=None, reps=1):
    if prep is None:
        prep = _prepare(inputs)
    prep["w2"] = np.asarray(inputs["W2"], np.float32).astype(np.float16)
    prep["w3"] = np.asarray(inputs["W3"], np.float32).astype(np.float16)
    prep["b2"] = np.asarray(inputs["b2"], np.float32).reshape(H2, 1)
    b3f = float(np.asarray(inputs["b3"], np.float32).reshape(-1)[0])
    nc = _build_program(prep["plan"], b3f, reps=reps)
    maps = _in_maps(prep)
    return {"nc": nc, "maps": maps, "prep": prep}


def _slot_of_out(C):
    """slot index for each element of the [4, C//4] device output."""
    cols = np.arange(C // 4)
    t = cols // (OUTTILE * GROUP // 4)
    rem = cols % (OUTTILE * GROUP // 4)
    k = rem // GROUP
    cc = rem % GROUP
    # group = t*OUTTILE + k*4 + q ; slot = group*GROUP + cc
    return ((t * OUTTILE + k * 4)[None, :] + np.arange(4)[:, None]) \
        * GROUP + cc[None, :]


def kernel(**inputs):
    prep = _prepare(inputs)
    built = _build(inputs, prep)
    res = run_bass_kernel_spmd(built["nc"], built["maps"],
                               list(range(NCORES)))

    C = prep["plan"]["C"]
    slot_of = _slot_of_out(C)
    pos2edge = prep["pos2edge"]
    out = np.zeros(E, np.float32)
    for c in range(NCORES):
        dev = np.asarray(res.results[c]["out"], np.float32)  # [4, C//4]
        full = np.empty(C, np.float32)
        full[slot_of.reshape(-1)] = dev.reshape(-1)
        m = pos2edge[c] >= 0
        out[pos2edge[c][m]] = full[m]

    fix = prep["fixup_edges"]
    if len(fix):
        emb = np.asarray(inputs["node_embeddings"], np.float32)
        W1 = np.asarray(inputs["W1"], np.float32)
        b1 = np.asarray(inputs["b1"], np.float32).reshape(-1)
        ei = np.asarray(inputs["edge_index"]).astype(np.int64)
        s, d = ei[0][fix], ei[1][fix]
        h = np.maximum(emb[s] @ W1[:D] + emb[d] @ W1[D:] + b1, 0.0)
        h = np.maximum(h @ np.asarray(inputs["W2"], np.float32)
                       + np.asarray(inputs["b2"], np.float32).reshape(-1),
                       0.0)
        out[fix] = (h @ np.asarray(inputs["W3"], np.float32)).reshape(-1) \
            + float(np.asarray(inputs["b3"], np.float32).reshape(-1)[0])
    return out.reshape(E, 1)


# revision 20
# speedup vs baseline: 3.3140x; 2.3951x over previous
"""LinkWeightDecoder Trainium2 kernel (v2).

out[e] = MLP(concat(emb[src[e]], emb[dst[e]])) for 1M edges over 8 cores.

Layer 1 is linear per endpoint, so per-node projections
  A1[u] = emb[u] @ W1[:D] + b1,   A2[u] = emb[u] @ W1[D:]
are precomputed per node (host, O(N*D*H1)) and stored f16. The device
computes out[e] = relu(relu(A1[src]+A2[dst]) @ W2 + b2) @ W3 + b3.

v2 design (vs v1's two per-edge gather sides): the HW floor is the SDMA
descriptor cost (~2.9 ns/desc measured, byte-count nearly irrelevant at
256B rows), so only the dst side pays per-edge descriptors:

- Edges shard by src block (12544 nodes/core), sort by (dst_bucket, src).
- src side has ZERO descriptors: for each 256-slot halfgroup the host
  streams a [64,128] stationary (the halfgroup's <=64 distinct nodes'
  A1 rows) plus a [64,256] one-hot routing matrix; one f16 matmul
  produces A1[src] feature-major in PSUM. 192B/slot of contiguous
  stream replaces a 2.9ns descriptor per slot.
- dst side: per-edge dma_gather of A2 rows (256B f16), 4 queues. int16
  indices are kept in range by 4 dst-bucket mega-runs per core (25000
  nodes/bucket), run capacities equalized across cores so all 8 cores
  share one program. Gathered edge-major tiles are transposed into the
  same PSUM group by f16 identity matmuls (regular matmul lhsT^T @ I,
  accumulating in f32).
- MLP: ACT relu -> h1 f16; W2 matmul; DVE fused (+b2, relu) -> h2 f16;
  W3 matmuls write [1,512] rows at PSUM partitions {0,32,64,96} via
  tile_position so output copies run 4 groups at a time.

Pad slots (run alignment + capacity equalization) gather row 0 / zero
one-hot columns and are dropped host-side via pos2edge.
"""
import math
import numpy as np

import concourse.bass as bass
import concourse.bacc as bacc
import concourse.mybir as mybir
import concourse.tile as tile
from concourse.bass_utils import run_bass_kernel_spmd

N = 100000
D = 128
E = 1000000
H1, H2 = 128, 64
NCORES = 8

NPC = 12544           # nodes per core (64-aligned, 8*NPC >= N)
DB = 25000            # dst bucket width (int16-safe indices)
NBUCK = 4
GROUP = 512           # slots per PSUM group
HG = 256              # slots per halfgroup (stationary unit)
BATCH = 2048          # slots per gather batch / gd tile
CALLMAX = 1024        # max idxs per dma_gather call
RUNALIGN = 128
OUTTILE = 8           # groups per output flush tile (4096 slots)
STREAMB = 2           # batches per stream DMA (4096 slots)
IDXB = 8              # batches per idx DMA

STB = 384             # stream cols per halfgroup: 128 stationary + 256 onehot

f32 = mybir.dt.float32
f16 = mybir.dt.float16
i16 = mybir.dt.int16

_AF = mybir.ActivationFunctionType
_ALU = mybir.AluOpType


def _wrap(vals):
    """[L] int16 -> [128, L//16]: pos i -> [i%16, i//16], replicated 8x
    down the partitions for the 8 Q7 cores."""
    w = vals.reshape(-1, 16).T
    return np.tile(w, (8, 1))


def _prepare(inputs):
    emb = np.asarray(inputs["node_embeddings"], np.float32)
    W1 = np.asarray(inputs["W1"], np.float32)
    b1 = np.asarray(inputs["b1"], np.float32).reshape(-1)
    a1 = (emb @ W1[:D] + b1).astype(np.float16)
    a2 = (emb @ W1[D:]).astype(np.float16)

    ei = np.asarray(inputs["edge_index"]).astype(np.int64)
    src, dst = ei[0], ei[1]
    core = np.minimum(src // NPC, NCORES - 1)

    # per-core edge lists sorted by (dst bucket, src)
    per_core = []
    counts = np.zeros((NCORES, NBUCK), np.int64)
    for c in range(NCORES):
        m = np.where(core == c)[0]
        es, ed = src[m], dst[m]
        bucket = ed // DB
        order = np.lexsort((es, bucket))
        m, es, ed, bucket = m[order], es[order], ed[order], bucket[order]
        for b in range(NBUCK):
            counts[c, b] = int((bucket == b).sum())
        per_core.append((m, es, ed, bucket))

    caps = [int(math.ceil(counts[:, b].max() / RUNALIGN) * RUNALIGN)
            for b in range(NBUCK)]
    C0 = sum(caps)
    C = int(math.ceil(C0 / (OUTTILE * GROUP)) * (OUTTILE * GROUP))
    tailpad = C - C0
    runs = [(sum(caps[:b]), caps[b], b) for b in range(NBUCK)]
    if tailpad:
        runs.append((C0, tailpad, 0))

    # slot arrays
    ssrc = np.full((NCORES, C), -1, np.int64)     # -1 = pad
    sdst16 = np.zeros((NCORES, C), np.int16)
    pos2edge = np.full((NCORES, C), -1, np.int64)
    for c in range(NCORES):
        m, es, ed, bucket = per_core[c]
        for b in range(NBUCK):
            lo = int(np.searchsorted(bucket, b))
            hi = int(np.searchsorted(bucket, b + 1))
            s0 = sum(caps[:b])
            n = hi - lo
            ssrc[c, s0:s0 + n] = es[lo:hi]
            sdst16[c, s0:s0 + n] = (ed[lo:hi] - b * DB).astype(np.int16)
            pos2edge[c, s0:s0 + n] = m[lo:hi]

    # gather call plan (common to all cores): (batch, off_in_batch, L, b)
    calls_by_batch = [[] for _ in range(C // BATCH)]
    for (r0, rlen, b) in runs:
        cur = r0
        end = r0 + rlen
        while cur < end:
            nb = (cur // BATCH + 1) * BATCH
            L = min(CALLMAX, end - cur, nb - cur)
            calls_by_batch[cur // BATCH].append((cur % BATCH, L, b))
            cur += L

    # idx image: global wrap of sdst16 (call slices line up since every
    # call offset is 128-aligned)
    gidx = np.zeros((NCORES, 128, C // 16), np.int16)
    for c in range(NCORES):
        gidx[c] = _wrap(sdst16[c])

    # stream image: per halfgroup [128, 128 stationary | 256 onehot].
    # Halfgroups with >128 distinct src nodes (rare) keep the 128 busiest
    # nodes; dropped slots are routed to the host fixup path.
    nhg = C // HG
    stream = np.zeros((NCORES, 128, nhg * STB), np.float16)
    fixup_edges = []
    for c in range(NCORES):
        sc = ssrc[c]
        for h in range(nhg):
            seg = sc[h * HG:(h + 1) * HG]
            valid = seg >= 0
            if not valid.any():
                continue
            nodes, inv, cnt = np.unique(seg[valid], return_inverse=True,
                                        return_counts=True)
            cols = np.nonzero(valid)[0]
            if len(nodes) > 128:
                keep = np.sort(np.argsort(-cnt, kind="stable")[:128])
                kept_mask = np.isin(inv, keep)
                drop_cols = cols[~kept_mask]
                drop_slots = h * HG + drop_cols
                fixup_edges.extend(pos2edge[c, drop_slots].tolist())
                pos2edge[c, drop_slots] = -1
                remap = -np.ones(len(nodes), np.int64)
                remap[keep] = np.arange(128)
                nodes = nodes[keep]
                inv = remap[inv]
                cols = cols[kept_mask]
                inv = inv[kept_mask]
            blk = stream[c, :, h * STB:(h + 1) * STB]
            blk[:len(nodes), :128] = a1[nodes]
            blk[inv, 128 + cols] = np.float16(1.0)

    plan = {"C": C, "calls_by_batch": calls_by_batch}
    return {"plan": plan, "gidx": gidx, "stream": stream,
            "pos2edge": pos2edge, "a2": a2,
            "fixup_edges": np.array(sorted(fixup_edges), np.int64)}


def _build_program(plan, b3f, reps=1, dbg_groups=0):
    nc = bacc.Bacc(num_swdge_queues=4)
    C = plan["C"]
    nhg = C // HG
    dbg_d = dbg2_d = None
    if dbg_groups:
        dbg_d = nc.dram_tensor("dbg", [128, dbg_groups * GROUP], f16,
                               kind="ExternalOutput")
        dbg2_d = nc.dram_tensor("dbg2", [H2, dbg_groups * GROUP], f16,
                                kind="ExternalOutput")
    a2t = nc.dram_tensor("a2t", [N, D], f16, kind="ExternalInput")
    gidx = nc.dram_tensor("gidx", [128, C // 16], i16, kind="ExternalInput")
    stream_d = nc.dram_tensor("stream", [128, nhg * STB], f16,
                              kind="ExternalInput")
    w2 = nc.dram_tensor("w2", [H1, H2], f16, kind="ExternalInput")
    w3 = nc.dram_tensor("w3", [H2, 1], f16, kind="ExternalInput")
    b2 = nc.dram_tensor("b2", [H2, 1], f32, kind="ExternalInput")
    ident = nc.dram_tensor("ident", [128, 128], f16, kind="ExternalInput")
    out_d = nc.dram_tensor("out", [4, C // 4], f16, kind="ExternalOutput")

    nbatch = C // BATCH
    g_per_b = BATCH // GROUP          # 4
    hg_per_g = GROUP // HG            # 2
    b_per_ot = OUTTILE * GROUP // BATCH   # 4 batches per outtile

    with tile.TileContext(nc) as tc:
        with (
            tc.tile_pool(name="const", bufs=1) as cpool,
            tc.tile_pool(name="idx", bufs=3) as ipool,
            tc.tile_pool(name="stm", bufs=3) as spool,
            tc.tile_pool(name="gd", bufs=6) as gpool,
            tc.tile_pool(name="h1", bufs=4) as h1pool,
            tc.tile_pool(name="h2", bufs=4) as h2pool,
            tc.tile_pool(name="osb", bufs=2) as opool,
            tc.tile_pool(name="pT", bufs=4, space="PSUM") as pTp,
            tc.tile_pool(name="p2p", bufs=2, space="PSUM") as p2p,
            tc.tile_pool(name="p3p", bufs=2, space="PSUM") as p3p,
        ):
            w2_t = cpool.tile([H1, H2], f16)
            w3_t = cpool.tile([H2, 1], f16)
            b2_t = cpool.tile([H2, 1], f32)
            id_t = cpool.tile([128, 128], f16)
            nc.sync.dma_start(out=w2_t[:], in_=w2[:, :])
            nc.sync.dma_start(out=w3_t[:], in_=w3[:, :])
            nc.sync.dma_start(out=b2_t[:], in_=b2[:, :])
            nc.sync.dma_start(out=id_t[:], in_=ident[:, :])

            qctr = 0
            for _ in range(reps):
                it = st = None
                state = {"outsb": None, "p3": None}
                b1q, b2q = [], []

                def stage_b1(gg, h1):
                    p2 = p2p.tile([H2, GROUP], f32, space="PSUM", tag="p2")
                    nc.tensor.matmul(out=p2[:], lhsT=w2_t[:], rhs=h1[:],
                                     start=True, stop=True)
                    h2s = h2pool.tile([H2, GROUP], f16, tag="h2")
                    nc.vector.tensor_scalar(
                        out=h2s[:], in0=p2[:], scalar1=b2_t[:],
                        scalar2=0.0, op0=_ALU.add, op1=_ALU.max)
                    if dbg2_d is not None and gg < dbg_groups:
                        nc.sync.dma_start(
                            out=dbg2_d[:, gg * GROUP:(gg + 1) * GROUP],
                            in_=h2s[:])
                    return h2s

                def stage_b2(gg, h2s):
                    q = gg % 4
                    if q == 0:
                        p3_t = p3p.tile([128, GROUP], f32, space="PSUM", tag="p3")
                        state["p3"] = p3_t
                    p3 = state["p3"]
                    nc.tensor.matmul(out=p3[32 * q:32 * q + 1, :],
                                     lhsT=w3_t[:], rhs=h2s[:],
                                     start=True, stop=True,
                                     tile_position=(0, 32 * q),
                                     skip_group_check=True)
                    if q == 3:
                        if state["outsb"] is None:
                            osb_t = opool.tile([128, OUTTILE * GROUP // 4], f16,
                                               tag="osb")
                            state["outsb"] = osb_t
                        k4 = (gg // 4) % (OUTTILE // 4)
                        nc.scalar.activation(
                            state["outsb"][:, k4 * GROUP:(k4 + 1) * GROUP],
                            p3[:], _AF.Copy, bias=b3f)
                        if k4 == OUTTILE // 4 - 1:
                            ot = gg // OUTTILE
                            ocols = OUTTILE * GROUP // 4
                            for qq in range(4):
                                nc.sync.dma_start(
                                    out=out_d[qq:qq + 1,
                                              ot * ocols:(ot + 1) * ocols],
                                    in_=state["outsb"][32 * qq:32 * qq + 1,
                                                       :])
                            state["outsb"] = None

                def pump(b1q, b2q):
                    if len(b2q) > 1:
                        gg2, h2s = b2q.pop(0)
                        stage_b2(gg2, h2s)
                    if len(b1q) > 1:
                        gg1, h1 = b1q.pop(0)
                        b2q.append((gg1, stage_b1(gg1, h1)))

                for bi in range(nbatch):
                    s0 = bi * BATCH
                    if bi % IDXB == 0:
                        icols = (min(C, s0 + IDXB * BATCH) - s0) // 16
                        it = ipool.tile([128, IDXB * BATCH // 16], i16,
                                        tag="it")
                        nc.sync.dma_start(
                            out=it[:, :icols],
                            in_=gidx[:, s0 // 16:s0 // 16 + icols])
                    if bi % STREAMB == 0:
                        scols = STREAMB * (BATCH // HG) * STB
                        soff = (s0 // HG) * STB
                        st = spool.tile([128, scols], f16, tag="st")
                        nc.sync.dma_start(
                            out=st[:], in_=stream_d[:, soff:soff + scols])
                    gd = gpool.tile([128, BATCH], f16, tag="gd")
                    gd3 = gd[:].rearrange("p (j f) -> p j f", f=D)
                    for (off, L, b) in plan["calls_by_batch"][bi]:
                        blo = b * DB
                        bhi = min(N, blo + DB)
                        icol = (s0 + off - (bi // IDXB) * IDXB * BATCH) // 16
                        nc.gpsimd.dma_gather(
                            out_ap=gd3[:, off // 128:(off + L) // 128, :],
                            in_ap=a2t[blo:bhi, :],
                            idxs_ap=it[:, icol:icol + L // 16],
                            num_idxs=L, num_idxs_reg=L, elem_size=D,
                            queue_num=qctr % 4,
                        )
                        qctr += 1

                    for g in range(g_per_b):
                        gg = bi * g_per_b + g          # global group idx
                        pT = pTp.tile([128, GROUP], f32, space="PSUM",
                                      tag="pT")
                        # src: one-hot matmuls, one per halfgroup
                        for k in range(hg_per_g):
                            hg = (s0 + g * GROUP) // HG + k
                            sb = (hg - (bi - bi % STREAMB) * BATCH // HG) \
                                * STB
                            nc.tensor.matmul(
                                out=pT[:, k * HG:(k + 1) * HG],
                                lhsT=st[:, sb:sb + 128],
                                rhs=st[:, sb + 128:sb + STB],
                                start=(k == 0), stop=False)
                        # dst: identity-matmul transposes, accumulate
                        for j in range(GROUP // 128):
                            blk = g * (GROUP // 128) + j
                            nc.tensor.matmul(
                                out=pT[:, j * 128:(j + 1) * 128],
                                lhsT=gd[:, blk * 128:(blk + 1) * 128],
                                rhs=id_t[:],
                                start=False, stop=(j == GROUP // 128 - 1))

                        h1 = h1pool.tile([128, GROUP], f16, tag="h1")
                        nc.scalar.activation(h1[:], pT[:], _AF.Relu)
                        if dbg_d is not None and gg < dbg_groups:
                            nc.sync.dma_start(
                                out=dbg_d[:, gg * GROUP:(gg + 1) * GROUP],
                                in_=h1[:])
                        b1q.append((gg, h1))
                        pump(b1q, b2q)

                # drain the software pipeline
                while b1q or b2q:
                    if b2q:
                        gg2, h2s = b2q.pop(0)
                        stage_b2(gg2, h2s)
                    if b1q:
                        gg1, h1 = b1q.pop(0)
                        b2q.append((gg1, stage_b1(gg1, h1)))

    nc.compile()
    return nc


def _in_maps(prep):
    base = {
        "a2t": np.ascontiguousarray(prep["a2"]),
        "w2": prep["w2"], "w3": prep["w3"], "b2": prep["b2"],
        "ident": np.eye(128, dtype=np.float16),
    }
    return [dict(base, gidx=np.ascontiguousarray(prep["gidx"][c]),
                 stream=np.ascontiguousarray(prep["stream"][c]))
            for c in range(NCORES)]


def _build(inputs, prep=None, reps=1):
    if prep is None:
        prep = _prepare(inputs)
    prep["w2"] = np.asarray(inputs["W2"], np.float32).astype(np.float16)
    prep["w3"] = np.asarray(inputs["W3"], np.float32).astype(np.float16)
    prep["b2"] = np.asarray(inputs["b2"], np.float32).reshape(H2, 1)
    b3f = float(np.asarray(inputs["b3"], np.float32).reshape(-1)[0])
    nc = _build_program(prep["plan"], b3f, reps=reps)
    maps = _in_maps(prep)
    return {"nc": nc, "maps": maps, "prep": prep}


def _slot_of_out(C):
    """slot index for each element of the [4, C//4] device output."""
    cols = np.arange(C // 4)
    t = cols // (OUTTILE * GROUP // 4)
    rem = cols % (OUTTILE * GROUP // 4)
    k = rem // GROUP
    cc = rem % GROUP
    # group = t*OUTTILE + k*4 + q ; slot = group*GROUP + cc
    return ((t * OUTTILE + k * 4)[None, :] + np.arange(4)[:, None]) \
        * GROUP + cc[None, :]


def kernel(**inputs):
    prep = _prepare(inputs)
    built = _build(inputs, prep)
    res = run_bass_kernel_spmd(built["nc"], built["maps"],
                               list(range(NCORES)))

    C = prep["plan"]["C"]
    slot_of = _slot_of_out(C)
    pos2edge = prep["pos2edge"]
    out = np.zeros(E, np.float32)
    for c in range(NCORES):
        dev = np.asarray(res.results[c]["out"], np.float32)  # [4, C//4]
        full = np.empty(C, np.float32)
        full[slot_of.reshape(-1)] = dev.reshape(-1)
        m = pos2edge[c] >= 0
        out[pos2edge[c][m]] = full[m]

    fix = prep["fixup_edges"]
    if len(fix):
        emb = np.asarray(inputs["node_embeddings"], np.float32)
        W1 = np.asarray(inputs["W1"], np.float32)
        b1 = np.asarray(inputs["b1"], np.float32).reshape(-1)
        ei = np.asarray(inputs["edge_index"]).astype(np.int64)
        s, d = ei[0][fix], ei[1][fix]
        h = np.maximum(emb[s] @ W1[:D] + emb[d] @ W1[D:] + b1, 0.0)
        h = np.maximum(h @ np.asarray(inputs["W2"], np.float32)
                       + np.asarray(inputs["b2"], np.float32).reshape(-1),
                       0.0)
        out[fix] = (h @ np.asarray(inputs["W3"], np.float32)).reshape(-1) \
            + float(np.asarray(inputs["b3"], np.float32).reshape(-1)[0])
    return out.reshape(E, 1)
